# revision 11
# baseline (speedup 1.0000x reference)
"""AFNO2D Trainium2 kernel: rfft2 -> block-diag complex MLP -> irfft2.

Sharding: one channel block (96 ch) per core x 8 cores. FFTs are per-channel
and the MLP is block-diagonal, so there is no cross-core communication.

All transforms are DFT matmuls (H=W=128 matches the PE array). Corner turns
between the W-axis and H-axis contractions go through a DRAM bounce + xbar
DMA transpose.
"""
import os
import sys
import numpy as np
import ml_dtypes

sys.path.insert(0, "/opt/trn_rl_repo")

B, H, W, C = 4, 128, 128, 768
NB, BS = 8, C // 8          # 8 blocks x 96 channels
WF = 65                     # rfft width
LAM = 0.01                  # softshrink
N = 128
SQ = float(np.sqrt(N))
BF16 = ml_dtypes.bfloat16

_CACHE = {}


def _dft_mats():
    idx = np.arange(N)
    ang = 2 * np.pi * np.outer(idx, idx) / N
    fw = np.zeros((N, N), np.float32)            # [w, kw_ri]
    fw[:, :65] = np.cos(ang[:, :65]) / SQ
    fw[:, 65:] = -np.sin(ang[:, 1:64]) / SQ
    gr = (np.cos(ang) / SQ).astype(np.float32)   # [h, kh]
    gi = (-np.sin(ang) / SQ).astype(np.float32)
    g = np.concatenate([gr, gi], axis=1)         # [h, 256]
    gn = np.concatenate([-gi, gr], axis=1)
    hc = (np.cos(ang) / SQ).astype(np.float32)   # [kh, h]
    hs = (np.sin(ang) / SQ).astype(np.float32)
    rw = np.zeros((N, N), np.float32)            # [kw_ri, w]
    wgt = np.full(65, 2.0, np.float32); wgt[0] = 1.0; wgt[64] = 1.0
    rw[:65, :] = (wgt[:, None] * np.cos(ang[:65, :])) / SQ
    rw[65:, :] = (-2.0 * np.sin(ang[1:64, :])) / SQ
    return fw, g, gn, hc, hs, (-hs).copy(), rw


def _build():
    from concourse import bass, bacc, tile, mybir

    bf = mybir.dt.bfloat16
    f32 = mybir.dt.float32
    AF = mybir.ActivationFunctionType
    ALU = mybir.AluOpType

    nc = bacc.Bacc("TRN2", target_bir_lowering=False, debug=False,
                   num_devices=8)

    x_d = nc.dram_tensor("x", [B, H, W, BS], f32, kind="ExternalInput").ap()
    fw_d = nc.dram_tensor("fw", [128, 128], bf, kind="ExternalInput").ap()
    g_d = nc.dram_tensor("g", [128, 256], bf, kind="ExternalInput").ap()
    gn_d = nc.dram_tensor("gn", [128, 256], bf, kind="ExternalInput").ap()
    hc_d = nc.dram_tensor("hc", [128, 128], bf, kind="ExternalInput").ap()
    hs_d = nc.dram_tensor("hs", [128, 128], bf, kind="ExternalInput").ap()
    hsn_d = nc.dram_tensor("hsn", [128, 128], bf, kind="ExternalInput").ap()
    rw_d = nc.dram_tensor("rw", [128, 128], bf, kind="ExternalInput").ap()
    w1r_d = nc.dram_tensor("w1r", [96, 96], bf, kind="ExternalInput").ap()
    w1i_d = nc.dram_tensor("w1i", [96, 96], bf, kind="ExternalInput").ap()
    w1in_d = nc.dram_tensor("w1in", [96, 96], bf, kind="ExternalInput").ap()
    b1_d = nc.dram_tensor("b1c", [96, 2], f32, kind="ExternalInput").ap()
    w2e_d = nc.dram_tensor("w2e", [97, 192], bf, kind="ExternalInput").ap()
    w2n_d = nc.dram_tensor("w2n", [96, 192], bf, kind="ExternalInput").ap()
    out_d = nc.dram_tensor("out", [B, H, W, BS], f32, kind="ExternalOutput").ap()

    with tile.TileContext(nc) as tc:
        from contextlib import ExitStack
        with ExitStack() as ctx:
            pconst = ctx.enter_context(tc.tile_pool(name="const", bufs=1))
            pin = ctx.enter_context(tc.tile_pool(name="pin", bufs=1))
            pspec = ctx.enter_context(tc.tile_pool(name="pspec", bufs=1))
            pspect = ctx.enter_context(tc.tile_pool(name="pspect", bufs=1))
            pbig = ctx.enter_context(tc.tile_pool(name="pbig", bufs=2))
            psml = ctx.enter_context(tc.tile_pool(name="psml", bufs=1))
            pout = ctx.enter_context(tc.tile_pool(name="pout", bufs=1))
            ptmp = ctx.enter_context(tc.tile_pool(name="ptmp", bufs=2))
            pps = ctx.enter_context(tc.tile_pool(name="pps", bufs=4, space="PSUM"))
            pdram = ctx.enter_context(tc.tile_pool(name="pdram", bufs=2, space="DRAM"))

            _cn = [0]
            def const(ap_d, shape, dtype=bf):
                _cn[0] += 1
                t = pconst.tile(list(shape), dtype, tag=f"const{_cn[0]}")
                nc.sync.dma_start(out=t[:], in_=ap_d)
                return t

            fw_s = const(fw_d, (128, 128))
            g_s = const(g_d, (128, 256))
            gn_s = const(gn_d, (128, 256))
            hc_s = const(hc_d, (128, 128))
            hs_s = const(hs_d, (128, 128))
            hsn_s = const(hsn_d, (128, 128))
            rw_s = const(rw_d, (128, 128))
            w1r_s = const(w1r_d, (96, 96))
            w1i_s = const(w1i_d, (96, 96))
            w1in_s = const(w1in_d, (96, 96))
            b1_s = const(b1_d, (96, 2), f32)
            w2e_s = const(w2e_d, (97, 192))
            w2n_s = const(w2n_d, (96, 192))
            neg_lam = pconst.tile([128, 1], f32)
            nc.gpsimd.memset(neg_lam[:], -LAM)

            for b in range(B):
                # ---- load + cast f32->bf16 in DMA: xb [w | h, c]
                xb = pin.tile([128, 128, BS], bf)
                for jh in range(2):
                    nc.gpsimd.dma_start(
                        out=xb[:, 64 * jh:64 * (jh + 1), :],
                        in_=x_d[b, 64 * jh:64 * (jh + 1)].rearrange("h w c -> w h c"))
                xb_f = xb[:].rearrange("w h c -> w (h c)")

                # ---- S1: rfft along W.  Y [kw_ri | c, h]
                y = pspec.tile([128, BS, 128], bf, tag="spec")
                for t in range(16):          # 16 groups x (2 mm of 384)
                    ps = pps.tile([128, 1024], f32, tag="ps")
                    for j in range(2):
                        sl = slice((2 * t + j) * 384, (2 * t + j + 1) * 384)
                        nc.tensor.matmul(ps[:, j * 512:j * 512 + 384], fw_s[:],
                                         xb_f[:, sl], start=True, stop=True)
                    h0 = t * 8
                    src = ps[:].rearrange("k (g x) -> k g x", g=2)[:, :, :384] \
                        .rearrange("k g (h c) -> k g h c", c=BS)
                    dst = y[:, :, h0:h0 + 8].rearrange("k c (g h) -> k g h c", g=2)
                    eng = nc.vector.tensor_copy if t % 2 == 0 else nc.scalar.copy
                    eng(dst, src)

                # ---- T1: DRAM bounce + xbar transpose -> yt [h | kw_ri, c]
                scr1 = pdram.tile([128, BS, 128], bf, tag="scr")
                nc.sync.dma_start(out=scr1[:], in_=y[:])
                yt = pspect.tile([128, 128, BS], bf, tag="spect")
                nc.sync.dma_start_transpose(
                    out=yt[:].rearrange("h k c -> h (k c)"),
                    in_=scr1[:].rearrange("k c h -> (k c) h"))

                # ---- S2: full DFT along H (complex).  X2 [c | ri, kw, kh]
                x2 = pbig.tile([96, 2, WF, 128], bf, tag="big")
                for gidx in range(17):       # groups of 4 kw
                    kws = list(range(4 * gidx, min(4 * gidx + 4, WF)))
                    if not kws:
                        break
                    ps = pps.tile([128, 1024], f32, tag="ps")
                    for j, kw in enumerate(kws):
                        o = ps[:96, j * 256:(j + 1) * 256]
                        single = kw in (0, 64)
                        nc.tensor.matmul(o, yt[:, kw, :], g_s[:],
                                         start=True, stop=single)
                        if not single:
                            nc.tensor.matmul(o, yt[:, 64 + kw, :], gn_s[:],
                                             start=False, stop=True)
                    src = ps[:96, :len(kws) * 256].rearrange(
                        "c (k r x) -> c k r x", k=len(kws), r=2)
                    dst = x2[:, :, kws[0]:kws[0] + len(kws), :].rearrange(
                        "c r k x -> c k r x")
                    eng = nc.vector.tensor_copy if gidx % 2 == 0 else nc.scalar.copy
                    eng(dst, src)

                x2r = x2[:, 0].rearrange("c k h -> c (k h)")   # [96 | 8320]
                x2i = x2[:, 1].rearrange("c k h -> c (k h)")

                # ---- MLP1 + gelu(+b1).  z [o(+ones) | ri, kw, kh]
                z = pbig.tile([97, 2, WF, 128], bf, tag="big")
                zr = z[0:96, 0].rearrange("c k h -> c (k h)")
                zi = z[0:96, 1].rearrange("c k h -> c (k h)")
                for q in range(9):           # 8 x 1024 + 128
                    n_tot = 1024 if q < 8 else 128
                    ps_r = pps.tile([128, 1024], f32, tag="ps")
                    ps_i = pps.tile([128, 1024], f32, tag="ps")
                    for sub in range(2 if q < 8 else 1):
                        nsz = min(512, n_tot - sub * 512)
                        sl = slice(q * 1024 + sub * 512, q * 1024 + sub * 512 + nsz)
                        pr = ps_r[:96, sub * 512:sub * 512 + nsz]
                        pi = ps_i[:96, sub * 512:sub * 512 + nsz]
                        nc.tensor.matmul(pr, w1r_s[:], x2r[:, sl], start=True, stop=False)
                        nc.tensor.matmul(pr, w1in_s[:], x2i[:, sl], start=False, stop=True)
                        nc.tensor.matmul(pi, w1i_s[:], x2r[:, sl], start=True, stop=False)
                        nc.tensor.matmul(pi, w1r_s[:], x2i[:, sl], start=False, stop=True)
                    osl = slice(q * 1024, q * 1024 + n_tot)
                    if q < 8:
                        srcr, srci = ps_r[:96, :], ps_i[:96, :]
                    else:
                        srcr, srci = ps_r[:96, :128], ps_i[:96, :128]
                    nc.scalar.activation(zr[:, osl], srcr, AF.Gelu, bias=b1_s[:, 0:1])
                    nc.scalar.activation(zi[:, osl], srci, AF.Gelu, bias=b1_s[:, 1:2])
                nc.gpsimd.memset(z[96:97, 0], 1.0)

                # ---- MLP2 (flipped: data as lhsT) + softshrink.  s [kh | ri, kw, c]
                s = psml.tile([128, 2, WF, 96], bf)
                for gidx in range(17):
                    kws = list(range(4 * gidx, min(4 * gidx + 4, WF)))
                    if not kws:
                        break
                    ps = pps.tile([128, 1024], f32, tag="ps")
                    for j, kw in enumerate(kws):
                        o = ps[:, j * 256:j * 256 + 192]
                        nc.tensor.matmul(o, z[0:97, 0, kw, :], w2e_s[:],
                                         start=True, stop=False)
                        nc.tensor.matmul(o, z[0:96, 1, kw, :], w2n_s[:],
                                         start=False, stop=True)
                    nk = len(kws)
                    psv = ps[:].rearrange("p (k x) -> p k x", k=4)[:, :nk, :192]
                    tA = ptmp.tile([128, 4, 192], bf, tag="tA")
                    tB = ptmp.tile([128, 4, 192], bf, tag="tB")
                    nc.vector.tensor_scalar(tA[:, :nk], psv, LAM, 0.0,
                                            op0=ALU.subtract, op1=ALU.max)
                    nc.scalar.activation(tB[:, :nk], psv, AF.Relu,
                                         bias=neg_lam[:], scale=-1.0)
                    dst = s[:, :, kws[0]:kws[0] + nk, :].rearrange("p r k c -> p k r c")
                    nc.vector.tensor_sub(dst,
                                         tA[:, :nk].rearrange("p k (r c) -> p k r c", r=2),
                                         tB[:, :nk].rearrange("p k (r c) -> p k r c", r=2))

                s_r = s[:, 0].rearrange("p k c -> p (k c)")    # [kh | 6240]
                s_i = s[:, 1].rearrange("p k c -> p (k c)")

                # ---- S5: inverse DFT along H.  hsb [h | c, kw_ri]
                hsb = pspec.tile([128, BS, 128], bf, tag="spec")
                for t in range(7):           # 6 x (2 x 480) + 1 x 480
                    nch = 2 if t < 6 else 1
                    ps_r = pps.tile([128, 1024], f32, tag="ps")
                    ps_i = pps.tile([128, 1024], f32, tag="ps")
                    for j in range(nch):
                        cidx = 2 * t + j
                        sl = slice(cidx * 480, (cidx + 1) * 480)
                        pr = ps_r[:, j * 512:j * 512 + 480]
                        pi = ps_i[:, j * 512:j * 512 + 480]
                        nc.tensor.matmul(pr, hc_s[:], s_r[:, sl], start=True, stop=False)
                        nc.tensor.matmul(pr, hsn_s[:], s_i[:, sl], start=False, stop=True)
                        nc.tensor.matmul(pi, hs_s[:], s_r[:, sl], start=True, stop=False)
                        nc.tensor.matmul(pi, hc_s[:], s_i[:, sl], start=False, stop=True)
                    kw0 = t * 10             # chunk cidx covers kw 5*cidx..+4
                    nkw = 10 if t < 6 else 5
                    # real part -> slots kw
                    srcr = ps_r[:].rearrange("p (g x) -> p g x", g=2)[:, :nch, :480] \
                        .rearrange("p g (k c) -> p g k c", c=96)
                    dstr = hsb[:, :, kw0:kw0 + nkw].rearrange("p c (g k) -> p g k c", g=nch)
                    eng = nc.vector.tensor_copy if t % 2 == 0 else nc.scalar.copy
                    eng(dstr, srcr)
                    # imag part -> slots 64+kw, dropping kw=0 and kw=64
                    if t == 0:
                        src_a = ps_i[:, 96:480].rearrange("p (k c) -> p k c", c=96)
                        dst_a = hsb[:, :, 65:69].rearrange("p c k -> p k c")
                        nc.scalar.copy(dst_a, src_a)
                        src_b = ps_i[:, 512:992].rearrange("p (k c) -> p k c", c=96)
                        dst_b = hsb[:, :, 69:74].rearrange("p c k -> p k c")
                        nc.vector.tensor_copy(dst_b, src_b)
                    elif t < 6:
                        srci = ps_i[:].rearrange("p (g x) -> p g x", g=2)[:, :, :480] \
                            .rearrange("p g (k c) -> p g k c", c=96)
                        dsti = hsb[:, :, 64 + kw0:64 + kw0 + 10].rearrange(
                            "p c (g k) -> p g k c", g=2)
                        eng2 = nc.scalar.copy if t % 2 == 0 else nc.vector.tensor_copy
                        eng2(dsti, srci)
                    else:
                        src_c = ps_i[:, 0:384].rearrange("p (k c) -> p k c", c=96)
                        dst_c = hsb[:, :, 124:128].rearrange("p c k -> p k c")
                        nc.scalar.copy(dst_c, src_c)

                # ---- T2: DRAM bounce + xbar transpose -> hst [kw_ri | h, c]
                scr2 = pdram.tile([128, BS, 128], bf, tag="scr")
                nc.sync.dma_start(out=scr2[:], in_=hsb[:])
                hst = pspect.tile([128, 128, BS], bf, tag="spect")
                nc.sync.dma_start_transpose(
                    out=hst[:].rearrange("k h c -> k (h c)"),
                    in_=scr2[:].rearrange("h c k -> (h c) k"))
                hst_f = hst[:].rearrange("k h c -> k (h c)")

                # ---- S6: irfft along W -> ob [w | h, c] -> DMA out (cast to f32)
                ob = pout.tile([128, 128, BS], bf)
                ob_f = ob[:].rearrange("w h c -> w (h c)")
                for t in range(12):          # 12 x 1024
                    ps = pps.tile([128, 1024], f32, tag="ps")
                    for j in range(2):
                        sl = slice(t * 1024 + j * 512, t * 1024 + (j + 1) * 512)
                        nc.tensor.matmul(ps[:, j * 512:(j + 1) * 512], rw_s[:],
                                         hst_f[:, sl], start=True, stop=True)
                    eng = nc.vector.tensor_copy if t % 2 == 0 else nc.scalar.copy
                    eng(ob_f[:, t * 1024:(t + 1) * 1024], ps[:])
                for jh in range(2):
                    nc.gpsimd.dma_start(
                        out=out_d[b, 64 * jh:64 * (jh + 1)].rearrange("h w c -> w h c"),
                        in_=ob[:, 64 * jh:64 * (jh + 1), :])

    nc.compile()
    return nc


def _prep_maps(x, w1, b1, w2, b2):
    fw, g, gn, hc, hs, hsn, rw = _dft_mats()
    shared = {
        "fw": fw.astype(BF16), "g": g.astype(BF16), "gn": gn.astype(BF16),
        "hc": hc.astype(BF16), "hs": hs.astype(BF16), "hsn": hsn.astype(BF16),
        "rw": rw.astype(BF16),
    }
    maps = []
    for n in range(NB):
        m = dict(shared)
        m["x"] = np.ascontiguousarray(x[:, :, :, n * BS:(n + 1) * BS]).astype(np.float32)
        m["w1r"] = w1[0, n].astype(BF16)
        m["w1i"] = w1[1, n].astype(BF16)
        m["w1in"] = (-w1[1, n]).astype(BF16)
        m["b1c"] = np.stack([b1[0, n], b1[1, n]], axis=1).astype(np.float32)
        w2e = np.zeros((97, 192), np.float32)
        w2e[:96, :96] = w2[0, n]
        w2e[:96, 96:] = w2[1, n]
        w2e[96, :96] = b2[0, n]
        w2e[96, 96:] = b2[1, n]
        m["w2e"] = w2e.astype(BF16)
        m["w2n"] = np.concatenate([-w2[1, n], w2[0, n]], axis=1).astype(BF16)
        maps.append(m)
    return maps


def _enable_trace():
    """Install the axon NTFF profile hook that the image's antenv lacks."""
    import types
    import importlib.util
    try:
        from antenv.axon_hooks import get_axon_ntff_profile_hook  # noqa: F401
        return
    except ImportError:
        pass
    spec = importlib.util.spec_from_file_location(
        "trn_boot_mod", "/root/.axon_site/trn_agent_boot/trn_boot.py")
    tb = importlib.util.module_from_spec(spec)
    spec.loader.exec_module(tb)
    hook = tb._ntff_profile_via_ctypes("/opt/axon/libaxon_pjrt.so")
    import antenv
    ah = types.ModuleType("antenv.axon_hooks")
    ah._hook = hook
    ah.get_axon_ntff_profile_hook = lambda: ah._hook
    ah.set_axon_ntff_profile_hook = lambda h: setattr(ah, "_hook", h)
    sys.modules["antenv.axon_hooks"] = ah
    antenv.axon_hooks = ah
    import concourse.bass_utils as bu
    bu.upload_artifacts = lambda tmpdir: "local://" + str(tmpdir)


def kernel(x, w1, b1, w2, b2, _trace=False):
    from concourse.bass_utils import run_bass_kernel_spmd

    if _trace:
        _enable_trace()
    if "nc" not in _CACHE:
        _CACHE["nc"] = _build()
    nc = _CACHE["nc"]
    maps = _prep_maps(np.asarray(x), np.asarray(w1), np.asarray(b1),
                      np.asarray(w2), np.asarray(b2))
    res = run_bass_kernel_spmd(nc, maps, core_ids=list(range(8)), trace=_trace)
    _CACHE["last_result"] = res
    out = np.concatenate([res.results[i]["out"] for i in range(8)], axis=3)
    return out.astype(np.float32)
